# revision 12
# baseline (speedup 1.0000x reference)
"""DCT2D kernel for Trainium2 (8 NeuronCores, SPMD data-parallel).

Math: per 8x8 block  out = scale * (C^T (x - 128) C)
  == out_flat[n, uv] = sum_xy o[n, xy] * W[xy, uv],  W = T * s,  o = x - 128.

The kernel is HBM/SDMA-traffic bound, so both directions are narrowed to
1 byte/element (12.6 MB/core vs 50.3 MB for fp32 in/out):

  in : 10 of 12 tiles carry o quantized to fp8 E3M4 (a native PE matmul
       dtype -- no conversion anywhere, scale 15.5/128 folded into a second
       fp16 weight set), the 6th carries round(o) as int8, cast to
       fp16 inside the SWDGE (gpsimd) DMA.  The mix balances SDMA bytes
       (fp8 tiles: 1 B/elem end-to-end; cast tiles: 1 B HBM, 2 B SBUF-side)
       against input quantization error (e3m4 1.21% / int8 0.39% rel fro);
       measured, a 1/6 cast fraction rides in otherwise-idle SDMA slack
       while more starts costing time.  0.5 MiB tiles (12/pass) interleave
       HBM reads and writes measurably better than 1 MiB tiles.
  out: PSUM fp32 is scaled by a per-coefficient quant scale and converted
       (round-to-nearest, saturating -- verified on HW) to int8 by the ACT
       and DVE engines, then DMAed out.  The host undoes the scale.

Output quant steps are ADAPTIVE: the host samples ~49k blocks of the real
input, computes their DCT, and sets step[uv] = (|mean|+5.25*sigma)/127 --
safe for any input distribution (the max over 786k samples of a
sub-Gaussian sum sits below 5.1 sigma; overflow merely saturates).
Total measured rel fro error ~1.5e-2 vs the 2e-2 gate.

Device layout: host packs each core's shard tile-major [ntiles, 128,
tile_f] int8 (fp8 tiles hold E3M4 bit patterns, bitcast on device), two
consecutive blocks stacked on partitions, weights blockdiag(W, W) [128,
128] fp16.  One matmul per 512 cols (PSUM bank), quantize in 1024-col
chunks alternating ACT/DVE, input/output DMAs spread across the sync,
scalar (HWDGE) and gpsimd (SWDGE) rings.
"""

import sys

if "/opt/trn_rl_repo" not in sys.path:
    sys.path.insert(0, "/opt/trn_rl_repo")

import numpy as np

import concourse.bass as bass  # noqa: F401
import concourse.mybir as mybir
import concourse.tile as tile
from concourse import bacc
from concourse.bass_utils import run_bass_kernel_spmd

N_CORES = 8
BLOCK = 8
B_DIM = 262144
C_DIM = 3
NBLK = B_DIM * C_DIM          # 786432 total 8x8 blocks
R = NBLK // N_CORES           # 98304 blocks per core
RP = R // 2                   # 49152 packed columns per core
TILE_F = 4096                 # columns per SBUF tile (0.5 MiB int8 per DMA)
NT = RP // TILE_F             # 6 tiles per pass
MM_F = 512                    # columns per matmul (one PSUM bank, fp32)
QCH = 1024                    # columns per quantize instruction (2 banks)

FP8_TILES = frozenset(t for t in range(12) if t not in (5, 11))  # rest: int8+cast
FP8_SCALE = 15.5 / 128.0           # o * FP8_SCALE fills the e3m4 range
K_SIGMA = 5.25                     # output quant range in sample sigmas

_CACHE = {}
last_results = None  # BassKernelResults of the most recent run (for test harness)

_F8NP = mybir.dt.np(mybir.dt.float8e3)


def _emit_pass(nc, xpool16, xpool8, ypool, pspool, w16_sb, w8_sb, q_sb, xt, out_t):
    f32 = mybir.dt.float32
    f16 = mybir.dt.float16
    f8 = mybir.dt.float8e3
    i8 = mybir.dt.int8
    for t in range(NT):
        if t in FP8_TILES:
            xr = xpool8.tile([128, TILE_F], i8, name="xr")
            nc.sync.dma_start(xr[:], xt[t])
            xin = xr.bitcast(f8)
            w_sb = w8_sb
        else:
            xin = xpool16.tile([128, TILE_F], f16, name="xin")
            nc.gpsimd.dma_start(xin[:], xt[t])  # int8 -> fp16 cast in-DMA
            w_sb = w16_sb
        yout = ypool.tile([128, TILE_F], i8, name="yout")
        for g in range(TILE_F // QCH):
            ps = pspool.tile([128, QCH], f32, name="ps")
            for m in range(QCH // MM_F):
                lo = g * QCH + m * MM_F
                nc.tensor.matmul(
                    ps[:, m * MM_F : (m + 1) * MM_F], w_sb[:],
                    xin[:, lo : lo + MM_F], start=True, stop=True,
                )
            dst = yout[:, g * QCH : (g + 1) * QCH]
            if g % 2 == 0:
                nc.scalar.activation(
                    dst, ps[:], mybir.ActivationFunctionType.Copy, scale=q_sb[:]
                )
            else:
                nc.vector.tensor_scalar_mul(dst, ps[:], q_sb[:])
        # Output ring phase matters: an out queued on the sync ring right
        # before the next input stalls it (HWDGE FIFO per ring) -- scalar
        # for odd tiles / sync for even measures ~4 us/pass faster than
        # the opposite phase.
        (nc.sync if t % 2 == 0 else nc.scalar).dma_start(out_t[t], yout[:])


def _build_nc(repeat=1):
    f32 = mybir.dt.float32
    f16 = mybir.dt.float16
    i8 = mybir.dt.int8
    nc = bacc.Bacc(None, target_bir_lowering=False, debug=False)
    xt = nc.declare_dram_parameter("xt", [NT, 128, TILE_F], i8, isOutput=False)
    w16 = nc.declare_dram_parameter("w16", [128, 128], f16, isOutput=False)
    w8 = nc.declare_dram_parameter("w8", [128, 128], f16, isOutput=False)
    qv = nc.declare_dram_parameter("qv", [128, 1], f32, isOutput=False)
    out = nc.declare_dram_parameter("out", [NT, 128, TILE_F], i8, isOutput=True)

    with tile.TileContext(nc) as tc:
        with (
            tc.tile_pool(name="consts", bufs=1) as cpool,
            tc.tile_pool(name="x16", bufs=4) as xpool16,
            tc.tile_pool(name="x8", bufs=12) as xpool8,
            tc.tile_pool(name="yout", bufs=12) as ypool,
            tc.tile_pool(name="ps", bufs=4, space="PSUM") as pspool,
        ):
            w16_sb = cpool.tile([128, 128], f16, name="w16_sb")
            nc.sync.dma_start(w16_sb[:], w16[:])
            w8_sb = cpool.tile([128, 128], f16, name="w8_sb")
            nc.sync.dma_start(w8_sb[:], w8[:])
            q_sb = cpool.tile([128, 1], f32, name="q_sb")
            nc.sync.dma_start(q_sb[:], qv[:])
            for _ in range(repeat):
                _emit_pass(nc, xpool16, xpool8, ypool, pspool,
                           w16_sb, w8_sb, q_sb, xt, out)
    nc.compile()
    return nc


def _consts(dct_tensor, scale):
    t_flat = np.asarray(dct_tensor, dtype=np.float64).reshape(64, 64)
    s_flat = np.asarray(scale, dtype=np.float64).reshape(64)
    w64 = t_flat * s_flat[None, :]
    w16 = np.zeros((128, 128), dtype=np.float16)
    w16[:64, :64] = w64
    w16[64:, 64:] = w64
    w8 = np.zeros((128, 128), dtype=np.float16)
    w8[:64, :64] = w64 / FP8_SCALE
    w8[64:, 64:] = w64 / FP8_SCALE
    return w16, w8, w64


def _adaptive_steps(xf, w64):
    """Per-coefficient int8 steps from a sample of the real data."""
    o_s = xf[:: max(1, NBLK // 49152)][:49152].astype(np.float64) - 128.0
    out_s = o_s @ w64
    mu = np.abs(out_s.mean(axis=0))
    sig = out_s.std(axis=0)
    steps = (mu + K_SIGMA * sig) / 127.0
    return np.maximum(steps, 1e-3).astype(np.float64)


def kernel(x, dct_tensor, scale):
    w16, w8, w64 = _consts(dct_tensor, scale)

    from concurrent.futures import ThreadPoolExecutor

    xf = np.asarray(x, dtype=np.float32).reshape(NBLK, 64)
    steps = _adaptive_steps(xf, w64)
    qv = np.concatenate([1.0 / steps, 1.0 / steps]).reshape(128, 1).astype(np.float32)
    steps_f32 = steps.astype(np.float32)

    def _pack(c):
        shard = xf[c * R : (c + 1) * R]
        o = shard.astype(np.float32) - 128.0
        xt = np.empty((NT, 128, TILE_F), np.int8)
        for t in range(NT):
            sub = o[t * 2 * TILE_F : (t + 1) * 2 * TILE_F]  # [2*TILE_F, 64]
            if t in FP8_TILES:
                enc = (sub * FP8_SCALE).astype(_F8NP).view(np.int8)
            else:
                enc = np.rint(sub).astype(np.int8)
            # xt[t, p*64+k, f] = enc[2f+p, k]
            xt[t] = (
                enc.reshape(TILE_F, 2, 64).transpose(1, 2, 0).reshape(128, TILE_F)
            )
        return xt

    with ThreadPoolExecutor(N_CORES) as pool:
        packs = list(pool.map(_pack, range(N_CORES)))
    in_maps = [{"xt": p, "w16": w16, "w8": w8, "qv": qv} for p in packs]

    if "nc" not in _CACHE:
        _CACHE["nc"] = _build_nc()
    res = run_bass_kernel_spmd(_CACHE["nc"], in_maps, core_ids=list(range(N_CORES)))
    global last_results
    last_results = res

    full = np.empty((NBLK, 64), dtype=np.float32)

    def _unpack(c):
        o = np.asarray(res.results[c]["out"])  # [NT, 128, TILE_F] int8 packed
        yi = o.reshape(NT, 2, 64, TILE_F).transpose(0, 3, 1, 2).reshape(R, 64)
        full[c * R : (c + 1) * R] = yi.astype(np.float32) * steps_f32[None, :]

    with ThreadPoolExecutor(N_CORES) as pool:
        list(pool.map(_unpack, range(N_CORES)))
    return full.reshape(B_DIM, C_DIM, BLOCK, BLOCK)


# revision 13
# speedup vs baseline: 1.0066x; 1.0066x over previous
"""DCT2D kernel for Trainium2 (8 NeuronCores, SPMD data-parallel).

Math: per 8x8 block  out = scale * (C^T (x - 128) C)
  == out_flat[n, uv] = sum_xy o[n, xy] * W[xy, uv],  W = T * s,  o = x - 128.

The kernel is HBM/SDMA-traffic bound, so both directions are narrowed to
1 byte/element (12.6 MB/core vs 50.3 MB for fp32 in/out):

  in : 10 of 12 tiles carry o quantized to fp8 E3M4 (a native PE matmul
       dtype -- no conversion anywhere, scale 15.5/128 folded into a second
       fp16 weight set), the 6th carries round(o) as int8, cast to
       fp16 inside the SWDGE (gpsimd) DMA.  The mix balances SDMA bytes
       (fp8 tiles: 1 B/elem end-to-end; cast tiles: 1 B HBM, 2 B SBUF-side)
       against input quantization error (e3m4 1.21% / int8 0.39% rel fro);
       measured, a 1/6 cast fraction rides in otherwise-idle SDMA slack
       while more starts costing time.  0.5 MiB tiles (12/pass) interleave
       HBM reads and writes measurably better than 1 MiB tiles.
  out: PSUM fp32 is scaled by a per-coefficient quant scale and converted
       (round-to-nearest, saturating -- verified on HW) to int8 by the ACT
       and DVE engines, then DMAed out.  The host undoes the scale.

Output quant steps are ADAPTIVE: the host samples ~49k blocks of the real
input, computes their DCT, and sets step[uv] = (|mean|+5.25*sigma)/127 --
safe for any input distribution (the max over 786k samples of a
sub-Gaussian sum sits below 5.1 sigma; overflow merely saturates).
Total measured rel fro error ~1.5e-2 vs the 2e-2 gate.

Device layout: host packs each core's shard tile-major [ntiles, 128,
tile_f] int8 (fp8 tiles hold E3M4 bit patterns, bitcast on device), two
consecutive blocks stacked on partitions, weights blockdiag(W, W) [128,
128] fp16.  One matmul per 512 cols (PSUM bank), quantize in 1024-col
chunks alternating ACT/DVE, input/output DMAs spread across the sync,
scalar (HWDGE) and gpsimd (SWDGE) rings.
"""

import sys

if "/opt/trn_rl_repo" not in sys.path:
    sys.path.insert(0, "/opt/trn_rl_repo")

import numpy as np

import concourse.bass as bass  # noqa: F401
import concourse.mybir as mybir
import concourse.tile as tile
from concourse import bacc
from concourse.bass_utils import run_bass_kernel_spmd

N_CORES = 8
BLOCK = 8
B_DIM = 262144
C_DIM = 3
NBLK = B_DIM * C_DIM          # 786432 total 8x8 blocks
R = NBLK // N_CORES           # 98304 blocks per core
RP = R // 2                   # 49152 packed columns per core
TILE_F = 4096                 # columns per SBUF tile (0.5 MiB int8 per DMA)
NT = RP // TILE_F             # 6 tiles per pass
MM_F = 512                    # columns per matmul (one PSUM bank, fp32)
QCH = 1024                    # columns per quantize instruction (2 banks)

FP8_TILES = frozenset(t for t in range(12) if t not in (5, 11))  # rest: int8+cast
FP8_SCALE = 15.5 / 128.0           # o * FP8_SCALE fills the e3m4 range
K_SIGMA = 5.25                     # output quant range in sample sigmas

_CACHE = {}
last_results = None  # BassKernelResults of the most recent run (for test harness)

_F8NP = mybir.dt.np(mybir.dt.float8e3)


def _emit_pass(nc, xpool16, xpool8, ypool, pspool, w16_sb, w8_sb, q_sb, xt, out_t):
    f32 = mybir.dt.float32
    f16 = mybir.dt.float16
    f8 = mybir.dt.float8e3
    i8 = mybir.dt.int8
    for t in range(NT):
        if t in FP8_TILES:
            xr = xpool8.tile([128, TILE_F], i8, name="xr")
            nc.sync.dma_start(xr[:], xt[t])
            xin = xr.bitcast(f8)
            w_sb = w8_sb
        else:
            xin = xpool16.tile([128, TILE_F], f16, name="xin")
            nc.gpsimd.dma_start(xin[:], xt[t])  # int8 -> fp16 cast in-DMA
            w_sb = w16_sb
        yout = ypool.tile([128, TILE_F], i8, name="yout")
        for g in range(TILE_F // QCH):
            ps = pspool.tile([128, QCH], f32, name="ps")
            for m in range(QCH // MM_F):
                lo = g * QCH + m * MM_F
                nc.tensor.matmul(
                    ps[:, m * MM_F : (m + 1) * MM_F], w_sb[:],
                    xin[:, lo : lo + MM_F], start=True, stop=True,
                )
            dst = yout[:, g * QCH : (g + 1) * QCH]
            if g % 2 == 0:
                nc.scalar.activation(
                    dst, ps[:], mybir.ActivationFunctionType.Copy, scale=q_sb[:]
                )
            else:
                nc.vector.tensor_scalar_mul(dst, ps[:], q_sb[:])
        # Output ring phase matters: an out queued on the sync ring right
        # before the next input stalls it (HWDGE FIFO per ring) -- scalar
        # for odd tiles / sync for even measures ~4 us/pass faster than
        # the opposite phase.
        (nc.sync if t % 2 == 0 else nc.scalar).dma_start(out_t[t], yout[:])


def _build_nc(repeat=1):
    f32 = mybir.dt.float32
    f16 = mybir.dt.float16
    i8 = mybir.dt.int8
    nc = bacc.Bacc(None, target_bir_lowering=False, debug=False)
    xt = nc.declare_dram_parameter("xt", [NT, 128, TILE_F], i8, isOutput=False)
    w16 = nc.declare_dram_parameter("w16", [128, 128], f16, isOutput=False)
    w8 = nc.declare_dram_parameter("w8", [128, 128], f16, isOutput=False)
    qv = nc.declare_dram_parameter("qv", [128, 1], f32, isOutput=False)
    out = nc.declare_dram_parameter("out", [NT, 128, TILE_F], i8, isOutput=True)

    with tile.TileContext(nc) as tc:
        with (
            tc.tile_pool(name="consts", bufs=1) as cpool,
            tc.tile_pool(name="x16", bufs=6) as xpool16,
            tc.tile_pool(name="x8", bufs=16) as xpool8,
            tc.tile_pool(name="yout", bufs=16) as ypool,
            tc.tile_pool(name="ps", bufs=4, space="PSUM") as pspool,
        ):
            w16_sb = cpool.tile([128, 128], f16, name="w16_sb")
            nc.sync.dma_start(w16_sb[:], w16[:])
            w8_sb = cpool.tile([128, 128], f16, name="w8_sb")
            nc.sync.dma_start(w8_sb[:], w8[:])
            q_sb = cpool.tile([128, 1], f32, name="q_sb")
            nc.sync.dma_start(q_sb[:], qv[:])
            for _ in range(repeat):
                _emit_pass(nc, xpool16, xpool8, ypool, pspool,
                           w16_sb, w8_sb, q_sb, xt, out)
    nc.compile()
    return nc


def _consts(dct_tensor, scale):
    t_flat = np.asarray(dct_tensor, dtype=np.float64).reshape(64, 64)
    s_flat = np.asarray(scale, dtype=np.float64).reshape(64)
    w64 = t_flat * s_flat[None, :]
    w16 = np.zeros((128, 128), dtype=np.float16)
    w16[:64, :64] = w64
    w16[64:, 64:] = w64
    w8 = np.zeros((128, 128), dtype=np.float16)
    w8[:64, :64] = w64 / FP8_SCALE
    w8[64:, 64:] = w64 / FP8_SCALE
    return w16, w8, w64


def _adaptive_steps(xf, w64):
    """Per-coefficient int8 steps from a sample of the real data."""
    o_s = xf[:: max(1, NBLK // 49152)][:49152].astype(np.float64) - 128.0
    out_s = o_s @ w64
    mu = np.abs(out_s.mean(axis=0))
    sig = out_s.std(axis=0)
    steps = (mu + K_SIGMA * sig) / 127.0
    return np.maximum(steps, 1e-3).astype(np.float64)


def kernel(x, dct_tensor, scale):
    w16, w8, w64 = _consts(dct_tensor, scale)

    from concurrent.futures import ThreadPoolExecutor

    xf = np.asarray(x, dtype=np.float32).reshape(NBLK, 64)
    steps = _adaptive_steps(xf, w64)
    qv = np.concatenate([1.0 / steps, 1.0 / steps]).reshape(128, 1).astype(np.float32)
    steps_f32 = steps.astype(np.float32)

    def _pack(c):
        shard = xf[c * R : (c + 1) * R]
        o = shard.astype(np.float32) - 128.0
        xt = np.empty((NT, 128, TILE_F), np.int8)
        for t in range(NT):
            sub = o[t * 2 * TILE_F : (t + 1) * 2 * TILE_F]  # [2*TILE_F, 64]
            if t in FP8_TILES:
                enc = (sub * FP8_SCALE).astype(_F8NP).view(np.int8)
            else:
                enc = np.rint(sub).astype(np.int8)
            # xt[t, p*64+k, f] = enc[2f+p, k]
            xt[t] = (
                enc.reshape(TILE_F, 2, 64).transpose(1, 2, 0).reshape(128, TILE_F)
            )
        return xt

    with ThreadPoolExecutor(N_CORES) as pool:
        packs = list(pool.map(_pack, range(N_CORES)))
    in_maps = [{"xt": p, "w16": w16, "w8": w8, "qv": qv} for p in packs]

    if "nc" not in _CACHE:
        _CACHE["nc"] = _build_nc()
    res = run_bass_kernel_spmd(_CACHE["nc"], in_maps, core_ids=list(range(N_CORES)))
    global last_results
    last_results = res

    full = np.empty((NBLK, 64), dtype=np.float32)

    def _unpack(c):
        o = np.asarray(res.results[c]["out"])  # [NT, 128, TILE_F] int8 packed
        yi = o.reshape(NT, 2, 64, TILE_F).transpose(0, 3, 1, 2).reshape(R, 64)
        full[c * R : (c + 1) * R] = yi.astype(np.float32) * steps_f32[None, :]

    with ThreadPoolExecutor(N_CORES) as pool:
        list(pool.map(_unpack, range(N_CORES)))
    return full.reshape(B_DIM, C_DIM, BLOCK, BLOCK)
